# revision 9
# baseline (speedup 1.0000x reference)
"""Trainium2 Bass kernel for nn_CrystallisationManager (vq_codebook).

One crystallisation step:
  velocity -> convergence counters -> snap-to-codebook (argmin over M) -> enforce.

Sharding: data-parallel over B=8 across the 8 NeuronCores (core b owns batch b).
Codebook-derived constants are replicated (host-precomputed, tiny).

Per-core layout (L=4096, H=16, M=64, d=64, DIM=1024):
  - loop over 32 L-tiles of 128 rows; each tile is [128, 1024] fp32.
  - velocity^2 per head via gpsimd sub + ACT square + DVE group-reduce.
  - scores g[l, m] = z.c_m - ||c_m||^2/2 per head via PE:
      PE transpose of z chunks (2 heads = 128 dims per chunk), fp32 matmul
      against block-diagonal transposed codebook, plus rank-1 (ones x -hn)
      accumulation in bf16 hi/lo.
  - argmax per head: DVE reduce_max + is_ge one-hot (exact-tie positions can
    only occur where `newly` is false, verified on the data distribution).
  - gather = one-hot matmul: PE transpose of the one-hot, then bf16 hi/lo
    block-diagonal codebook matmul -> entries[l, d] (fp32-exact selection).
  - masked merges via DVE copy_predicated (in-place into frozen/z tiles).
"""

import sys

for _p in ("/opt/pypackages", "/opt/trn_rl_repo"):
    if _p not in sys.path:
        sys.path.append(_p)

import numpy as np
import ml_dtypes
from contextlib import ExitStack

import concourse.mybir as mybir
import concourse.bacc as bacc
import concourse.tile as tile
from concourse import masks
from concourse.bass_utils import run_bass_kernel_spmd

F32 = mybir.dt.float32
BF16 = mybir.dt.bfloat16
U8 = mybir.dt.uint8
OP = mybir.AluOpType
ACTF = mybir.ActivationFunctionType

B, L, DIM = 8, 4096, 1024
H, M, D = 16, 64, 64
N_CORES = 8
P = 128                      # partition rows per L-tile
NT = L // P                  # 32 L-tiles per core
NCHUNK = DIM // 128          # 8 chunks of 2 heads each

_PROG = None  # cached (nc, names)


def _vel2_threshold():
    """fp32 t2 with (v < t2)  <=>  (sqrt32(v) < fp32(0.01))."""
    c0 = np.float32(0.01)
    v = np.float32(np.float64(c0) ** 2)
    one = np.float32(1)
    while np.float32(np.sqrt(v)) >= c0:
        v = np.nextafter(v, np.float32(0))
    while np.float32(np.sqrt(v)) < c0:
        v = np.nextafter(v, np.float32(np.inf))
    return float(v)


def _build_program():
    nc = bacc.Bacc("TRN2", target_bir_lowering=False, debug=False,
                   num_devices=N_CORES)

    def din(name, shape, dt):
        return nc.dram_tensor(name, shape, dt, kind="ExternalInput").ap()

    def dout(name, shape, dt):
        return nc.dram_tensor(name, shape, dt, kind="ExternalOutput").ap()

    zc_d = din("zc", [L, DIM], F32)
    zp_d = din("zp", [L, DIM], F32)
    fr_d = din("fr", [L, DIM], F32)
    cc_d = din("cc", [L, H], F32)
    cy_d = din("cy", [L, H], F32)
    cbT_d = din("cbT", [128, DIM], F32)      # blockdiag cb^T per chunk [d2, m2]
    cbhi_d = din("cbhi", [128, DIM], BF16)   # blockdiag cb per chunk  [m2, d2]
    cblo_d = din("cblo", [128, DIM], BF16)
    hnhi_d = din("hnhi", [1, DIM], BF16)     # -||c||^2/2 rows
    hnlo_d = din("hnlo", [1, DIM], BF16)

    zenf_d = dout("zenf", [L, DIM], F32)
    frnew_d = dout("frnew", [L, DIM], F32)
    ccn_d = dout("ccn", [L, H], F32)
    nwl_d = dout("nwl", [L, H], U8)
    cry_d = dout("cry", [L, H], U8)

    t2 = _vel2_threshold()

    with tile.TileContext(nc) as tc, ExitStack() as ctx:
        const = ctx.enter_context(tc.tile_pool(name="const", bufs=1))
        io = ctx.enter_context(tc.tile_pool(name="io", bufs=3))
        wk = ctx.enter_context(tc.tile_pool(name="wk", bufs=2))
        sm = ctx.enter_context(tc.tile_pool(name="sm", bufs=3))
        ps_zT = ctx.enter_context(tc.tile_pool(name="ps_zT", bufs=1, space="PSUM"))
        ps_g = ctx.enter_context(tc.tile_pool(name="ps_g", bufs=1, space="PSUM"))
        ps_oh = ctx.enter_context(tc.tile_pool(name="ps_oh", bufs=2, space="PSUM"))
        ps_ent = ctx.enter_context(tc.tile_pool(name="ps_ent", bufs=1, space="PSUM"))

        ident_f = const.tile([128, 128], F32)
        ident_b = const.tile([128, 128], BF16)
        masks.make_identity(nc, ident_f[:])
        masks.make_identity(nc, ident_b[:])
        ones_b = const.tile([1, 128], BF16)
        nc.gpsimd.memset(ones_b[:], 1.0)

        cbT_s = const.tile([128, DIM], F32)
        cbhi_s = const.tile([128, DIM], BF16)
        cblo_s = const.tile([128, DIM], BF16)
        hnhi_s = const.tile([1, DIM], BF16)
        hnlo_s = const.tile([1, DIM], BF16)
        nc.sync.dma_start(out=cbT_s[:], in_=cbT_d[:])
        nc.sync.dma_start(out=cbhi_s[:], in_=cbhi_d[:])
        nc.sync.dma_start(out=cblo_s[:], in_=cblo_d[:])
        nc.sync.dma_start(out=hnhi_s[:], in_=hnhi_d[:])
        nc.sync.dma_start(out=hnlo_s[:], in_=hnlo_d[:])

        for lt in range(NT):
            rows = slice(lt * P, (lt + 1) * P)

            zc_t = io.tile([P, DIM], F32, tag="zc")
            zp_t = io.tile([P, DIM], F32, tag="zp")
            fr_t = io.tile([P, DIM], F32, tag="fr")
            cc_t = sm.tile([P, H], F32, tag="cc")
            cy_t = sm.tile([P, H], F32, tag="cy")
            nc.sync.dma_start(out=zc_t[:], in_=zc_d[rows, :])
            nc.sync.dma_start(out=zp_t[:], in_=zp_d[rows, :])
            nc.sync.dma_start(out=fr_t[:], in_=fr_d[rows, :])
            nc.sync.dma_start(out=cc_t[:], in_=cc_d[rows, :])
            nc.sync.dma_start(out=cy_t[:], in_=cy_d[rows, :])

            # ---- velocity^2 per head -> convergence masks ----
            diff = wk.tile([P, DIM], F32, tag="diff")
            nc.any.tensor_tensor(diff[:], zc_t[:], zp_t[:], OP.subtract)
            sq = wk.tile([P, DIM], F32, tag="sq")
            nc.scalar.activation(sq[:], diff[:], ACTF.Square)
            vel2 = sm.tile([P, H], F32, tag="vel2")
            nc.vector.tensor_reduce(
                vel2[:], sq[:].rearrange("p (h d) -> p h d", h=H),
                axis=mybir.AxisListType.X, op=OP.add)

            conv = sm.tile([P, H], F32, tag="conv")
            nc.any.tensor_scalar(conv[:], vel2[:], t2, None, OP.is_lt)
            cc1 = sm.tile([P, H], F32, tag="cc1")
            nc.any.tensor_scalar(cc1[:], cc_t[:], 1.0, None, OP.add)
            ccn_t = sm.tile([P, H], F32, tag="ccn")
            nc.any.tensor_tensor(ccn_t[:], cc1[:], conv[:], OP.mult)
            nw0 = sm.tile([P, H], F32, tag="nw0")
            nc.any.tensor_scalar(nw0[:], ccn_t[:], 2.0, None, OP.is_ge)
            nwl_u8 = sm.tile([P, H], U8, tag="nwlu")
            nc.any.tensor_tensor(nwl_u8[:], nw0[:], cy_t[:], OP.is_gt)
            cry_u8 = sm.tile([P, H], U8, tag="cryu")
            nc.any.tensor_tensor(cry_u8[:], cy_t[:], nwl_u8[:], OP.max)

            nc.sync.dma_start(out=ccn_d[rows, :], in_=ccn_t[:])
            nc.sync.dma_start(out=nwl_d[rows, :], in_=nwl_u8[:])
            nc.sync.dma_start(out=cry_d[rows, :], in_=cry_u8[:])

            # ---- transpose z chunks: [128 l, 128 d] -> [128 d, 128 l] ----
            # PSUM `start=True` zeroes the whole 2 KiB bank (zero region), so
            # exactly one start per bank (4 fp32 chunks per bank) and one stop
            # on the bank's last write.
            zT_ps = ps_zT.tile([P, DIM], F32, tag="zT_ps")
            for c in range(NCHUNK):
                cs = slice(c * 128, (c + 1) * 128)
                nc.tensor.matmul(zT_ps[:, cs], zc_t[:, cs], ident_f[:],
                                 is_transpose=True,
                                 start=(c % 4 == 0), stop=(c % 4 == 3))
            zT_s = wk.tile([P, DIM], F32, tag="zT_s")
            nc.scalar.copy(zT_s[:], zT_ps[:])

            # ---- scores: g[l, m] = z.c - hn  (fp32 matmul + bf16 hi/lo rank-1)
            g_ps = ps_g.tile([P, DIM], F32, tag="g_ps")
            for c in range(NCHUNK):
                cs = slice(c * 128, (c + 1) * 128)
                nc.tensor.matmul(g_ps[:, cs], zT_s[:, cs], cbT_s[:, cs],
                                 start=(c % 4 == 0), stop=False)
            for c in range(NCHUNK):
                cs = slice(c * 128, (c + 1) * 128)
                nc.tensor.matmul(g_ps[:, cs], ones_b[:], hnhi_s[:, cs],
                                 start=False, stop=False)
                nc.tensor.matmul(g_ps[:, cs], ones_b[:], hnlo_s[:, cs],
                                 start=False, stop=(c % 4 == 3))

            # ---- per-head argmax -> one-hot (bf16) ----
            mx = sm.tile([P, H], F32, tag="mx")
            nc.vector.tensor_reduce(
                mx[:], g_ps[:].rearrange("p (h m) -> p h m", h=H),
                axis=mybir.AxisListType.X, op=OP.max)
            oh_s = wk.tile([P, DIM], BF16, tag="oh_s")
            nc.vector.tensor_tensor(
                oh_s[:].rearrange("p (h m) -> p h m", h=H),
                g_ps[:].rearrange("p (h m) -> p h m", h=H),
                mx[:, :, None].to_broadcast([P, H, M]),
                OP.is_ge)

            # ---- transpose one-hot, gather entries = onehot @ cb ----
            # oh_ps is bf16 [128, 1024] = 2 KiB/partition = ONE psum bank.
            oh_ps = ps_oh.tile([P, DIM], BF16, tag="oh_ps")
            for c in range(NCHUNK):
                cs = slice(c * 128, (c + 1) * 128)
                nc.tensor.matmul(oh_ps[:, cs], oh_s[:, cs], ident_b[:],
                                 is_transpose=True,
                                 start=(c == 0), stop=(c == NCHUNK - 1))
            ohT_s = wk.tile([P, DIM], BF16, tag="ohT_s")
            nc.scalar.copy(ohT_s[:], oh_ps[:])

            ent_ps = ps_ent.tile([P, DIM], F32, tag="ent_ps")
            for c in range(NCHUNK):
                cs = slice(c * 128, (c + 1) * 128)
                nc.tensor.matmul(ent_ps[:, cs], ohT_s[:, cs], cbhi_s[:, cs],
                                 start=(c % 4 == 0), stop=False)
                nc.tensor.matmul(ent_ps[:, cs], ohT_s[:, cs], cblo_s[:, cs],
                                 start=False, stop=(c % 4 == 3))

            # ---- masked merges (in place) ----
            nc.vector.copy_predicated(
                fr_t[:].rearrange("p (h d) -> p h d", h=H),
                nwl_u8[:, :, None].to_broadcast([P, H, D]),
                ent_ps[:].rearrange("p (h d) -> p h d", h=H))
            nc.vector.copy_predicated(
                zc_t[:].rearrange("p (h d) -> p h d", h=H),
                cry_u8[:, :, None].to_broadcast([P, H, D]),
                fr_t[:].rearrange("p (h d) -> p h d", h=H))

            nc.sync.dma_start(out=frnew_d[rows, :], in_=fr_t[:])
            nc.sync.dma_start(out=zenf_d[rows, :], in_=zc_t[:])

    nc.compile()
    return nc


def _get_prog():
    global _PROG
    if _PROG is None:
        _PROG = _build_program()
    return _PROG


def _host_constants(codebook):
    cb = np.asarray(codebook, dtype=np.float32)          # [H, M, D]
    cbT = np.zeros((128, NCHUNK, 128), np.float32)
    cbb = np.zeros((128, NCHUNK, 128), np.float64)
    for c in range(NCHUNK):
        cbT[0:64, c, 0:64] = cb[2 * c].T
        cbT[64:128, c, 64:128] = cb[2 * c + 1].T
        cbb[0:64, c, 0:64] = cb[2 * c]
        cbb[64:128, c, 64:128] = cb[2 * c + 1]
    cbhi = cbb.astype(ml_dtypes.bfloat16)
    cblo = (cbb - cbhi.astype(np.float64)).astype(ml_dtypes.bfloat16)

    hn = -0.5 * (cb.astype(np.float64) ** 2).sum(-1)     # [H, M]
    hnr = np.zeros((1, NCHUNK, 128), np.float64)
    for c in range(NCHUNK):
        hnr[0, c, 0:64] = hn[2 * c]
        hnr[0, c, 64:128] = hn[2 * c + 1]
    hnhi = hnr.astype(ml_dtypes.bfloat16)
    hnlo = (hnr - hnhi.astype(np.float64)).astype(ml_dtypes.bfloat16)

    return {
        "cbT": cbT.reshape(128, DIM),
        "cbhi": cbhi.reshape(128, DIM),
        "cblo": cblo.reshape(128, DIM),
        "hnhi": hnhi.reshape(1, DIM),
        "hnlo": hnlo.reshape(1, DIM),
    }


def kernel(z_current, z_prev, codebook, frozen_values, consecutive_converged,
           crystallised):
    nc = _get_prog()
    consts = _host_constants(codebook)

    zc = np.ascontiguousarray(np.asarray(z_current, dtype=np.float32))
    zp = np.ascontiguousarray(np.asarray(z_prev, dtype=np.float32))
    fr = np.ascontiguousarray(
        np.asarray(frozen_values, dtype=np.float32).reshape(B, L, DIM))
    cc = np.asarray(consecutive_converged).astype(np.float32)
    cy = np.asarray(crystallised).astype(np.float32)

    in_maps = []
    for b in range(N_CORES):
        m = {"zc": zc[b], "zp": zp[b], "fr": fr[b], "cc": cc[b], "cy": cy[b]}
        m.update(consts)
        in_maps.append(m)

    res = run_bass_kernel_spmd(nc, in_maps, list(range(N_CORES)))
    r = res.results

    z_enforced = np.stack([r[b]["zenf"] for b in range(B)]).reshape(B, L, DIM)
    frozen_new = np.stack([r[b]["frnew"] for b in range(B)]).reshape(B, L, H, D)
    cc_new = np.rint(np.stack([r[b]["ccn"] for b in range(B)])).astype(np.int32)
    newly = np.stack([r[b]["nwl"] for b in range(B)]) != 0
    crystal_new = np.stack([r[b]["cry"] for b in range(B)]) != 0

    return z_enforced, crystal_new, newly, cc_new, frozen_new
